# revision 7
# baseline (speedup 1.0000x reference)
"""Trainium2 Bass kernel for CoordLSVotingWeighted (segment_reduce).

Strategy: data-parallel over batch B=8 across 8 NeuronCores (1 image/core).

Per image, on device (single full-image pass):
  - hard one-hot of argmax over 9 seg channels (matches softmax(seg*1e6))
  - unit-direction projection features via a custom fused DVE op:
      rinv = approx 1/(nx^2+ny^2)   (bitwise-NOT seed + 1 Newton step)
      t = softplus(w)*rinv ; u = t*nx ; m = u*ny ; R11 = u*nx
    softplus on the scalar engine (Exp then Ln, one ACT table set);
    R00 is never materialized: R00 = sp - R11, recovered on host from
    the sp-feature accumulators.
  - segment reduce on TensorE, G=4 w-columns packed per matmul:
      lhsT = interleaved L [w, {hot, hot*ch, hot*cw}, class]  (96 cols)
      rhs  = planar R [w-window, {sp, m, R11}, point]         (108 cols)
      PSUM [96, 108] accumulates; diagonal 24x27 blocks summed on host.
  - inputs in 3 contiguous bf16 DRAM tensors, row-split into 6 dma_starts
    across the sync and gpsimd queues for parallel descriptor generation
    and more DMA-queue parallelism
  - wide PE warmup matmuls keep the tensor engine's pstate up until the
    real accumulation windows are ready
Host: assemble 2x2 systems in float64, pinv-solve, scale by HEIGHT.

Self-contained: only needs numpy / ml_dtypes / concourse (installed env).
"""

import os

import numpy as np

B = 8
H = 128
W = 128
NCLS = 9  # seg channels, class 0 = background
NPTS = 9
OC = 8
HEIGHT = 128.0
N_CORES = 8

G = 4            # w columns packed per matmul
NF = NPTS * W    # 1152 point-cols

SEG_C = 2 * W * NCLS       # seg f32 as bf16 cols
WCV_C = NF + W * OC + 2    # w | cwb | chv

N_WARM = int(os.environ.get("KERNEL_WARM", "20"))
WARM_N = 512  # moving cols per warmup matmul

# 1-Newton reciprocal-approx constants (minimax over s in [1e-8, 1e8])
RC0 = -0.2355
RC1 = 2.0015

_cache: dict = {}


def _register_rinv():
    """Runtime-register the custom DVE op RINV_XY = recip1(x^2 + y^2)."""
    import concourse.dve_ops as dops
    from concourse.dve_spec import (
        Spec, Src0, Src1, C0, C1, AluOp, Bin, lower, _has_src1,
    )
    from concourse.dve_uop import DveOpSpec

    for o in dops.OPS:
        if o.name == "RINV_XY":
            return o

    s = Src0 * Src0 + Src1 * Src1
    nb = Bin(AluOp.BITWISE_NOT, s, s)
    y0 = nb * C0
    y1 = y0 * (C1 - s * y0)

    def _ref(in0, in1, s0, s1, imm2):
        ss = (in0.astype(np.float32) ** 2 + in1.astype(np.float32) ** 2).astype(
            np.float32
        )
        nbv = (~ss.view(np.int32)).view(np.float32)
        y0v = (nbv * np.float32(s0)).astype(np.float32)
        return (y0v * (np.float32(s1) - ss * y0v)).astype(np.float32)

    spec = Spec(body=y1, reference=_ref)
    opcode = dops._CUSTOM_DVE_ROW_BASE + len(dops.OPS)
    shas = {}
    for ver in ("v3", "v4"):
        try:
            shas[ver] = DveOpSpec(
                name="RINV_XY",
                opcode=opcode,
                uops=lower(spec, ver=ver),
                rd1_en=_has_src1(spec),
            ).sha(ver)
        except Exception:
            pass
    op = dops.DveOp("RINV_XY", spec, subdim=False, uops_sha=shas)
    dops.OPS.append(op)
    dops.CUSTOM_DVE_SPECS[op.name] = op.spec
    dops._SUB_OPCODE_FOR_NAME[op.name] = opcode
    return op


def _patch_act_tables():
    """Exp and Ln resolve only to natural_log_exp_and_others -> 1 table load."""
    import concourse.bacc as bacc
    import concourse.mybir as mybir

    A = mybir.ActivationFunctionType
    orig = bacc.get_activation_tables
    if getattr(orig, "_softplus_patched", False):
        return

    def patched(arch):
        out = {}
        for name, funcs in orig(arch).items():
            f = set(funcs)
            if name != "natural_log_exp_and_others":
                f.discard(A.Exp)
                f.discard(A.Ln)
            out[name] = f
        return out

    patched._softplus_patched = True
    bacc.get_activation_tables = patched


def _build_nc():
    import concourse.bacc as bacc
    import concourse.tile as tile
    import concourse.mybir as mybir
    from concourse.alu_op_type import AluOpType as Alu

    Act = mybir.ActivationFunctionType
    Axis = mybir.AxisListType
    f32 = mybir.dt.float32
    b16 = mybir.dt.bfloat16

    RINV = _register_rinv()
    _patch_act_tables()

    nc = bacc.Bacc(
        "TRN2", target_bir_lowering=False, debug=False, num_devices=N_CORES
    )
    seg_d = nc.dram_tensor("seg", [H, SEG_C], b16, kind="ExternalInput")
    nyx_d = nc.dram_tensor("nyx", [H, 2 * NF], b16, kind="ExternalInput")
    wcv_d = nc.dram_tensor("wcv", [H, WCV_C], b16, kind="ExternalInput")
    out_d = nc.dram_tensor("acc", [G * 24, G * 27], f32, kind="ExternalOutput")

    HH = H // 2

    with tile.TileContext(nc) as tc:
        with (
            tc.tile_pool(name="main", bufs=1) as pool,
            tc.tile_pool(name="ps", bufs=1, space="PSUM") as psp,
        ):
            acc = psp.tile([G * 24, G * 27], f32, tag="acc")
            wmL = pool.tile([H, 2 * G * 24], b16, tag="wmL")
            wmR = pool.tile([H, WARM_N], b16, tag="wmR")
            wacc = psp.tile([G * 24, WARM_N], f32, tag="wacc")

            sgt = pool.tile([H, SEG_C], b16, tag="sgt")
            nyxt = pool.tile([H, 2 * NF], b16, tag="nyxt")
            wcvt = pool.tile([H, WCV_C], b16, tag="wcvt")

            # input DMAs: row-halves, spread over sync and gpsimd queues
            nc.sync.dma_start(out=wcvt[0:HH, :], in_=wcv_d[0:HH, :])
            nc.gpsimd.dma_start(out=wcvt[HH:H, :], in_=wcv_d[HH:H, :])
            nc.sync.dma_start(out=nyxt[0:HH, :], in_=nyx_d[0:HH, :])
            nc.gpsimd.dma_start(out=nyxt[HH:H, :], in_=nyx_d[HH:H, :])
            nc.sync.dma_start(out=sgt[0:HH, :], in_=seg_d[0:HH, :])
            nc.gpsimd.dma_start(out=sgt[HH:H, :], in_=seg_d[HH:H, :])

            # warmup matmuls keep PE pstate up while DMA + DVE run
            nc.vector.memset(wmL[:, :], 0)
            nc.vector.memset(wmR[:, :], 0)
            for i in range(N_WARM):
                nc.tensor.matmul(
                    wacc[:, :],
                    wmL[:, (i % 2) * G * 24 : (i % 2 + 1) * G * 24],
                    wmR[:, :],
                    start=True,
                    stop=True,
                )

            sgf = sgt[:, :].bitcast(f32)
            nyv = nyxt[:, 0:NF]
            nxv = nyxt[:, NF : 2 * NF]
            wtv = wcvt[:, 0:NF]
            cwv = wcvt[:, NF : NF + W * OC]
            cht = wcvt[:, NF + W * OC : NF + W * OC + 2].bitcast(f32)

            mxt = pool.tile([H, W], f32, tag="mxt")
            ewt = pool.tile([H, NF], b16, tag="ewt")
            rit = pool.tile([H, NF], b16, tag="rit")
            tt = pool.tile([H, NF], b16, tag="tt")
            ut = pool.tile([H, NF], b16, tag="ut")
            L = pool.tile([H, W * 3 * OC], b16, tag="L")
            R = pool.tile([H, 3 * NF], b16, tag="R")
            L4 = L[:, :].rearrange("q (w f k) -> q w f k", f=3, k=OC)

            # ---- scalar chain: softplus -> sp (R feature block 0)
            nc.scalar.activation(out=ewt[:, :], in_=wtv, func=Act.Exp)
            nc.scalar.activation(
                out=R[:, 0:NF], in_=ewt[:, :], func=Act.Ln, bias=1.0
            )

            # ---- vector chain (single in-order stream)
            nc.vector._custom_dve(
                RINV, out=rit[:, :], in0=nxv, in1=nyv, s0=RC0, s1=RC1
            )
            sg_wc = sgf.rearrange("q (w k) -> q w k", k=NCLS)
            nc.vector.tensor_reduce(
                out=mxt[:, :], in_=sg_wc, axis=Axis.X, op=Alu.max
            )
            mx_b = mxt[:, :].unsqueeze(2).broadcast_to((H, W, OC))
            hot = L4[:, :, 0, :]
            nc.vector.tensor_tensor(
                out=hot, in0=sg_wc[:, :, 1:NCLS], in1=mx_b, op=Alu.is_equal
            )
            nc.vector.tensor_tensor(
                out=tt[:, :], in0=R[:, 0:NF], in1=rit[:, :], op=Alu.mult
            )
            nc.vector.tensor_tensor(
                out=ut[:, :], in0=tt[:, :], in1=nxv, op=Alu.mult
            )
            u_b = ut[:, :].unsqueeze(1).broadcast_to((H, 2, NF))
            nyx_r = nyxt[:, :].rearrange("q (b f) -> q b f", b=2)
            mr_out = R[:, NF : 3 * NF].rearrange("q (b f) -> q b f", b=2)
            nc.vector.tensor_tensor(out=mr_out, in0=u_b, in1=nyx_r, op=Alu.mult)
            cw_r = cwv.rearrange("q (w k) -> q w k", k=OC)
            nc.vector.tensor_tensor(
                out=L4[:, :, 2, :], in0=hot, in1=cw_r, op=Alu.mult
            )
            # hotch on the scalar engine (per-partition scale)
            nc.scalar.mul(out=L4[:, :, 1, :], in_=hot, mul=cht)

            # ---- segment reduce on TensorE, G columns per matmul
            Rv = R[:, :].rearrange("q (f g w) -> q w f g", f=3, g=NPTS)
            nwin = W // G
            for wi in range(nwin):
                nc.tensor.matmul(
                    acc[:, :],
                    L[:, wi * G * 24 : (wi + 1) * G * 24],
                    Rv[:, wi * G : (wi + 1) * G, :, :],
                    start=(wi == 0),
                    stop=(wi == nwin - 1),
                )

            outs = pool.tile([G * 24, G * 27], f32, tag="outs")
            nc.scalar.copy(out=outs[:, :], in_=acc[:, :])
            nc.scalar.dma_start(out=out_d[:, :], in_=outs[:, :])

    nc.compile()
    return nc


def _prep_inputs(seg, direct, w):
    """Host-side sharding/staging: dtype cast + layout permutation only."""
    import ml_dtypes

    bf16 = ml_dtypes.bfloat16
    segc = np.ascontiguousarray(seg.reshape(B, H, W * NCLS)).view(bf16)
    d = direct.reshape(B, H, W, NPTS, 2)
    nyx = (
        np.ascontiguousarray(d.transpose(0, 1, 4, 3, 2)[:, :, ::-1, :, :])
        .astype(bf16)
        .reshape(B, H, 2 * NF)
    )
    wb = (
        np.ascontiguousarray(w.reshape(B, H, W, NPTS).transpose(0, 1, 3, 2))
        .astype(bf16)
        .reshape(B, H, NF)
    )
    cw = ((np.arange(W, dtype=np.float32) + 0.5) / HEIGHT).astype(bf16)
    cwb = np.ascontiguousarray(
        np.broadcast_to(cw.reshape(1, W, 1), (H, W, OC))
    ).reshape(1, H, W * OC)
    cwb = np.broadcast_to(cwb, (B, H, W * OC))
    chv = (
        ((np.arange(H, dtype=np.float32) + 0.5) / HEIGHT)
        .reshape(H, 1)
        .view(bf16)
        .reshape(1, H, 2)
    )
    chv = np.broadcast_to(chv, (B, H, 2))
    wcv = np.concatenate([wb, cwb, chv], axis=2)
    assert wcv.shape == (B, H, WCV_C)
    return segc, nyx, np.ascontiguousarray(wcv)


def _solve_host(a96: np.ndarray) -> np.ndarray:
    """acc [96,108] fp32 -> p [OC, NPTS, 2] fp32 (float64 pinv like ref)."""
    a = a96.astype(np.float64)
    acc = np.zeros((24, 27), dtype=np.float64)
    for j in range(G):
        acc += a[j * 24 : (j + 1) * 24, j * 27 : (j + 1) * 27]
    H0, H1, H2 = acc[0:OC], acc[OC : 2 * OC], acc[2 * OC : 3 * OC]
    SP0, M0, D0 = H0[:, 0:9], H0[:, 9:18], H0[:, 18:27]
    SP1, M1, D1 = H1[:, 0:9], H1[:, 9:18], H1[:, 18:27]
    SP2, M2, D2 = H2[:, 0:9], H2[:, 9:18], H2[:, 18:27]
    A = SP0 - D0
    Bm = M0
    D = D0
    qx = (SP1 - D1) - M2
    qy = D2 - M1
    Rm = np.empty((OC, NPTS, 2, 2), dtype=np.float64)
    Rm[..., 0, 0] = A
    Rm[..., 0, 1] = -Bm
    Rm[..., 1, 0] = -Bm
    Rm[..., 1, 1] = D
    q = np.stack([qx, qy], axis=-1)
    Rp = np.linalg.pinv(Rm.reshape(-1, 2, 2)).reshape(Rm.shape)
    p = np.einsum("cpij,cpj->cpi", Rp, q) * HEIGHT
    return p.astype(np.float32)


def kernel(seg, direct, w):
    if "nc" not in _cache:
        _cache["nc"] = _build_nc()
    nc = _cache["nc"]

    seg = np.ascontiguousarray(np.asarray(seg, dtype=np.float32))
    direct = np.ascontiguousarray(np.asarray(direct, dtype=np.float32))
    w = np.ascontiguousarray(np.asarray(w, dtype=np.float32))
    segc, nyx, wcv = _prep_inputs(seg, direct, w)

    in_maps = []
    for i in range(B):
        in_maps.append({"seg": segc[i], "nyx": nyx[i], "wcv": wcv[i]})

    from concourse.bass_utils import run_bass_kernel_spmd

    trace = bool(int(os.environ.get("KERNEL_TRACE", "0")))
    res = run_bass_kernel_spmd(
        nc, in_maps, core_ids=list(range(N_CORES)), trace=trace
    )
    kernel._last_exec_ns = res.exec_time_ns
    kernel._last_results = res

    out = np.stack(
        [_solve_host(np.asarray(res.results[i]["acc"])) for i in range(B)], axis=0
    )
    return out
